# revision 11
# baseline (speedup 1.0000x reference)
"""L2SquaredConv2d (1x1 conv) on 8 TRN2 NeuronCores — fp8 DoubleRow version.

out[b,p,h,w] = relu( sum_c x^2  - 2*sum_c x*w[p]  + sum_c w[p]^2 )

Strategy: data-parallel over batch (B=32 -> 4 images/core, N=3136 pixels).
Per core one [P=2000, C=512] x [C, N] matmul done in fp8 (e4m3) with
perf_mode=DoubleRow (2 k-subtiles of 128 per MM -> K=256 per MM, 2x stream
rate vs bf16; warm cadence 189ns per FD=448 MM, stationary load hidden).
The -2 is folded into the fp8 weights on the host.  The relu is dropped:
the true output is sum_c (x-w)^2 >= 0 by construction, so relu = identity.

DMA rings: one HWDGE ring (sync) moves ~200KB at a time (~324GB/s per
transfer, serial per ring), so inputs go on the sync ring and outputs go on
the GPSIMD SWDGE ring (gpsimd engine is otherwise idle).  The per-core
output is written as ONE [M,3136] DMA per p-chunk into a [P, BL*HW]-layout
dram tensor (6272B contiguous lines); the host transposes back to
[BL, P, HW].

Per p-chunk (16, 7 n-chunks of FD=448, PSUM = 8-slot ring of single banks):
  pass1: 7 DR MMs with w[k0:2] (start)  pass2: 7 DR MMs with w[k2:4] (stop)
  A: evict psum -> u (bf16, +w2[p]): chunk c0 on DVE (tensor_scalar_add),
     chunks c1-c6 on ScalarE (2x activation Identity, bias=w2[p])
  B: o = u + i2r (DVE tensor_tensor bf16 2x, one op per p-chunk)
  one gpsimd-ring DMA out.

Prologue: i2r[128,N] built once: x^2 from fp8 x (chunks c0-c3 on ScalarE
Square, c4-c6 on DVE mul) -> ones-matmul broadcast into psum slots 0-6 ->
copies to sbuf (scalar).  w2[p] = sum_c w^2 via DVE
scalar_tensor_tensor(w*w, accum_out), spread across the main loop.
"""

import numpy as np
import ml_dtypes

import concourse.bacc as bacc
import concourse.bass as bass
import concourse.mybir as mybir
import concourse.tile as tile
from concourse import bass_utils

B, C, H, W = 32, 512, 28, 28
P = 2000
NCORES = 8
BL = B // NCORES          # 4 images per core
HW = H * W                # 784
N = BL * HW               # 3136 pixels per core
KC = C // 128             # 4 contraction subtiles
FD = 448                  # matmul moving free dim (psum chunk width)
NCH = N // FD             # 7 chunks per p-chunk
PC = (P + 127) // 128     # 16 p-chunks (last one is 80 rows)
P_PAD = PC * 128
DVE_CH = 1                # eviction chunks handled by DVE (rest on ScalarE)

BF16 = mybir.dt.bfloat16
FP8 = mybir.dt.float8e4
F32 = mybir.dt.float32
NPBF16 = ml_dtypes.bfloat16
NPFP8 = ml_dtypes.float8_e4m3
DR = mybir.MatmulPerfMode.DoubleRow

_CACHE = {}


def _runs(slots):
    """Split a list of ring slots into maximal consecutive runs.
    Returns list of (start_slot, length, offset_in_list)."""
    runs = []
    i = 0
    while i < len(slots):
        j = i
        while j + 1 < len(slots) and slots[j + 1] == slots[j] + 1:
            j += 1
        runs.append((slots[i], j - i + 1, i))
        i = j + 1
    return runs


def _build():
    nc = bacc.Bacc(
        "TRN2", target_bir_lowering=False, debug=False, num_devices=NCORES
    )
    x8_d = nc.dram_tensor("x8", [KC, 128, N], FP8, kind="ExternalInput")
    w8_d = nc.dram_tensor("w8", [KC, 128, P], FP8, kind="ExternalInput")
    wpc_d = nc.dram_tensor("w_pc", [PC, 128, C], BF16, kind="ExternalInput")
    out_d = nc.dram_tensor("out", [P, N], BF16, kind="ExternalOutput")
    ones_d = nc.inline_tensor(np.ones((128, 128), dtype=NPBF16), "ones_mat")

    IDENT = mybir.ActivationFunctionType.Identity
    SQUARE = mybir.ActivationFunctionType.Square
    MULT = mybir.AluOpType.mult

    with tile.TileContext(nc) as tc:
        with (
            tc.tile_pool(name="resident", bufs=1) as rpool,
            tc.tile_pool(name="x2p", bufs=2) as x2_pool,
            tc.tile_pool(name="sqp", bufs=2) as sq_pool,
            tc.tile_pool(name="up", bufs=2) as u_pool,
            tc.tile_pool(name="op", bufs=3) as o_pool,
            tc.tile_pool(name="pm", bufs=1, space=bass.MemorySpace.PSUM) as pm_pool,
        ):
            # ---- resident tiles ----
            x8_sb = rpool.tile([128, KC, N], FP8, tag="x8sb", name="x8sb")
            w8_sb = rpool.tile([128, KC, P], FP8, tag="w8sb", name="w8sb")
            ones_sb = rpool.tile([128, 128], BF16, tag="ones", name="ones_sb")
            i2r = rpool.tile([128, N], BF16, tag="i2r", name="i2r")
            w2col = rpool.tile([128, PC], F32, tag="w2col", name="w2col")
            wpc_t = [
                rpool.tile([128, C], BF16, tag=f"wpc{i}", name=f"wpc{i}")
                for i in range(PC)
            ]

            # whole PSUM as one 8-bank ring; each slot = one bank, FD used
            ps_all = pm_pool.tile(
                [128, 8, FD], F32, padded_shape=[128, 8, 512], name="ps_all"
            )

            # ---- input DMAs, all on the sync HWDGE ring (the only ring
            # whose issuing engine is otherwise idle; a DMA instruction
            # blocks its engine's queue for the whole transfer).
            # x8 first third leads so the squares start ASAP. ----
            nc.sync.dma_start(ones_sb[:], ones_d[:])
            T0, T1 = 1024, 2048
            for k in range(KC):
                nc.sync.dma_start(x8_sb[:, k, 0:T0], x8_d[k, :, 0:T0])
            for k in range(2):
                nc.sync.dma_start(w8_sb[:, k, :], w8_d[k])
            for k in range(KC):
                nc.sync.dma_start(x8_sb[:, k, T0:T1], x8_d[k, :, T0:T1])
            for i in range(2):
                nc.sync.dma_start(wpc_t[i][:], wpc_d[i])
            for k in range(2, KC):
                nc.sync.dma_start(w8_sb[:, k, :], w8_d[k])
            for k in range(KC):
                nc.sync.dma_start(x8_sb[:, k, T1:N], x8_d[k, :, T1:N])
            for i in range(2, PC):
                nc.sync.dma_start(wpc_t[i][:], wpc_d[i])

            def w2_op(i):
                sq = sq_pool.tile([128, C], BF16, tag="sq", name="sq")
                nc.vector.scalar_tensor_tensor(
                    sq[:], wpc_t[i][:], 1.0, wpc_t[i][:],
                    op0=MULT, op1=MULT,
                    accum_out=w2col[:, i:i + 1],
                )

            # ---- PE warm-up: dummy matmuls fill the boilerplate-to-data
            # gap and flip the HAM clock gate to 2.4GHz before real work ----
            for _ in range(48):
                nc.tensor.matmul(
                    ps_all[:, 7, 0:128], ones_sb[:], ones_sb[:],
                    start=True, stop=True,
                )

            # ---- i2 phase: x^2 + ones-matmul broadcast, slots 0..6 ----
            for c in range(NCH):
                sl = slice(FD * c, FD * (c + 1))
                x2t = x2_pool.tile([128, KC, FD], BF16, tag="x2t", name="x2t")
                if c < 4:
                    nc.scalar.activation(x2t[:], x8_sb[:, :, sl], SQUARE)
                else:
                    nc.vector.tensor_mul(
                        x2t[:], x8_sb[:, :, sl], x8_sb[:, :, sl]
                    )
                for k in range(KC):
                    nc.tensor.matmul(
                        ps_all[:, c, :], ones_sb[:], x2t[:, k, :],
                        start=(k == 0), stop=(k == KC - 1),
                    )
                nc.scalar.copy(i2r[:, sl], ps_all[:, c, :])

            w2_op(0)
            w2_op(1)

            # ---- main loop over p-chunks ----
            for pc in range(PC):
                M = min(128, P - 128 * pc)
                psl = slice(128 * pc, 128 * pc + M)
                slots = [(NCH + NCH * pc + t) % 8 for t in range(NCH)]

                # pass1/pass2: fp8 DoubleRow accumulate of -2*x.w
                for t in range(NCH):
                    nsl = slice(FD * t, FD * (t + 1))
                    nc.tensor.matmul(
                        ps_all[:M, slots[t], :],
                        w8_sb[:, 0:2, psl], x8_sb[:, 0:2, nsl],
                        start=True, stop=False, perf_mode=DR,
                    )
                for t in range(NCH):
                    nsl = slice(FD * t, FD * (t + 1))
                    nc.tensor.matmul(
                        ps_all[:M, slots[t], :],
                        w8_sb[:, 2:4, psl], x8_sb[:, 2:4, nsl],
                        start=False, stop=True, perf_mode=DR,
                    )

                # spread the w2 producers across the loop
                if pc + 2 < PC:
                    w2_op(pc + 2)

                u = u_pool.tile([128, N], BF16, tag="u", name="u")
                o = o_pool.tile([128, N], BF16, tag="o", name="o")
                u3 = u.rearrange("p (c f) -> p c f", f=FD)

                # A: evict psum -> u (+w2[p]).  c0 on DVE; c1..c6 ScalarE
                # (last pc: DVE takes c0-c3 so the tail drains in parallel)
                dve_ch = DVE_CH if pc < PC - 1 else 4
                for s0, ln, off in _runs(slots[0:dve_ch]):
                    nc.vector.tensor_scalar_add(
                        u3[:M, off:off + ln, :],
                        ps_all[:M, s0:s0 + ln, :],
                        w2col[:M, pc:pc + 1],
                    )
                for grp_base, grp_end in ((dve_ch, 4), (4, NCH)):
                    grp = slots[grp_base:grp_end]
                    for s0, ln, off in _runs(grp):
                        off += grp_base
                        nc.scalar.activation(
                            u3[:M, off:off + ln, :],
                            ps_all[:M, s0:s0 + ln, :],
                            IDENT, bias=w2col[:M, pc:pc + 1], scale=1.0,
                        )

                # B: o = u + i2r (bf16 2x); out-DMA on the gpsimd ring
                # (6272B lines).  Last pc: split both so the tail pipelines.
                if pc < PC - 1:
                    nc.vector.tensor_add(o[:M, :], u[:M, :], i2r[:M, :])
                    nc.gpsimd.dma_start(out_d[psl, :], o[:M, :])
                else:
                    HN = 4 * FD
                    nc.vector.tensor_add(
                        o[:M, 0:HN], u[:M, 0:HN], i2r[:M, 0:HN]
                    )
                    nc.gpsimd.dma_start(out_d[psl, 0:HN], o[:M, 0:HN])
                    nc.vector.tensor_add(
                        o[:M, HN:N], u[:M, HN:N], i2r[:M, HN:N]
                    )
                    nc.gpsimd.dma_start(out_d[psl, HN:N], o[:M, HN:N])

    nc.compile()
    return nc


def _get_nc():
    if "nc" not in _CACHE:
        _CACHE["nc"] = _build()
    return _CACHE["nc"]


def _make_in_maps(input, weights):
    x = np.asarray(input, dtype=np.float32)
    w = np.asarray(weights, dtype=np.float32).reshape(P, C)

    wT = np.ascontiguousarray((-2.0 * w).T)               # [C, P]
    w8 = wT.astype(NPFP8).reshape(KC, 128, P)
    w_pad = np.zeros((P_PAD, C), np.float32)
    w_pad[:P] = w
    w_pc = w_pad.astype(NPBF16).reshape(PC, 128, C)

    in_maps = []
    for c in range(NCORES):
        sh = x[c * BL:(c + 1) * BL]                       # [4, 512, 28, 28]
        xT = np.ascontiguousarray(sh.transpose(1, 0, 2, 3).reshape(C, N))
        x8 = xT.astype(NPFP8).reshape(KC, 128, N)
        in_maps.append({"x8": x8, "w8": w8, "w_pc": w_pc})
    return in_maps


def run(input, weights, trace=False):
    """Returns (output [32,2000,28,28] f32, BassKernelResults)."""
    nc = _get_nc()
    in_maps = _make_in_maps(input, weights)
    res = bass_utils.run_bass_kernel_spmd(
        nc, in_maps, core_ids=list(range(NCORES)), trace=trace
    )
    # per-core out is [P, N] = [P, BL, HW]; reorder to [BL, P, HW]
    outs = [
        res.results[c]["out"].reshape(P, BL, HW).transpose(1, 0, 2)
        for c in range(NCORES)
    ]
    out = (
        np.concatenate(outs, axis=0).astype(np.float32).reshape(B, P, H, W)
    )
    return out, res


def kernel(input, weights):
    out, _ = run(input, weights, trace=False)
    return out


# revision 12
# speedup vs baseline: 1.1862x; 1.1862x over previous
"""L2SquaredConv2d (1x1 conv) on 8 TRN2 NeuronCores — fp8 DoubleRow version.

out[b,p,h,w] = relu( sum_c x^2  - 2*sum_c x*w[p]  + sum_c w[p]^2 )

Strategy: data-parallel over batch (B=32 -> 4 images/core, N=3136 pixels).
Per core one [P=2000, C=512] x [C, N] matmul done in fp8 (e4m3) with
perf_mode=DoubleRow (2 k-subtiles of 128 per MM -> K=256 per MM, 2x stream
rate vs bf16; warm cadence 189ns per FD=448 MM, stationary load hidden).
The -2 is folded into the fp8 weights on the host.  The relu is dropped:
the true output is sum_c (x-w)^2 >= 0 by construction, so relu = identity.

DMA rings: one HWDGE ring (sync) moves ~200KB at a time (~324GB/s per
transfer, serial per ring), so inputs go on the sync ring and outputs go on
the GPSIMD SWDGE ring (gpsimd engine is otherwise idle).  The per-core
output is written as ONE [M,3136] DMA per p-chunk into a [P, BL*HW]-layout
dram tensor (6272B contiguous lines); the host transposes back to
[BL, P, HW].

Per p-chunk (16, 7 n-chunks of FD=448, PSUM = 8-slot ring of single banks):
  pass1: 7 DR MMs with w[k0:2] (start)  pass2: 7 DR MMs with w[k2:4] (stop)
  A: evict psum -> u (bf16, +w2[p]): chunk c0 on DVE (tensor_scalar_add),
     chunks c1-c6 on ScalarE (2x activation Identity, bias=w2[p])
  B: o = u + i2r (DVE tensor_tensor bf16 2x, one op per p-chunk)
  one gpsimd-ring DMA out.

Prologue: i2r[128,N] built once: x^2 from fp8 x (chunks c0-c3 on ScalarE
Square, c4-c6 on DVE mul) -> ones-matmul broadcast into psum slots 0-6 ->
copies to sbuf (scalar).  w2[p] = sum_c w^2 via DVE
scalar_tensor_tensor(w*w, accum_out), spread across the main loop.
"""

import numpy as np
import ml_dtypes

import concourse.bacc as bacc
import concourse.bass as bass
import concourse.mybir as mybir
import concourse.tile as tile
from concourse import bass_utils

B, C, H, W = 32, 512, 28, 28
P = 2000
NCORES = 8
BL = B // NCORES          # 4 images per core
HW = H * W                # 784
N = BL * HW               # 3136 pixels per core
KC = C // 128             # 4 contraction subtiles
FD = 448                  # matmul moving free dim (psum chunk width)
NCH = N // FD             # 7 chunks per p-chunk
PC = (P + 127) // 128     # 16 p-chunks (last one is 80 rows)
P_PAD = PC * 128
DVE_CH = 1                # eviction chunks handled by DVE (rest on ScalarE)

BF16 = mybir.dt.bfloat16
FP8 = mybir.dt.float8e4
F32 = mybir.dt.float32
NPBF16 = ml_dtypes.bfloat16
NPFP8 = ml_dtypes.float8_e4m3
DR = mybir.MatmulPerfMode.DoubleRow

_CACHE = {}


def _runs(slots):
    """Split a list of ring slots into maximal consecutive runs.
    Returns list of (start_slot, length, offset_in_list)."""
    runs = []
    i = 0
    while i < len(slots):
        j = i
        while j + 1 < len(slots) and slots[j + 1] == slots[j] + 1:
            j += 1
        runs.append((slots[i], j - i + 1, i))
        i = j + 1
    return runs


def _build():
    nc = bacc.Bacc(
        "TRN2", target_bir_lowering=False, debug=False, num_devices=NCORES
    )
    x8_d = nc.dram_tensor("x8", [KC, 128, N], FP8, kind="ExternalInput")
    w8_d = nc.dram_tensor("w8", [KC, 128, P], FP8, kind="ExternalInput")
    wpc_d = nc.dram_tensor("w_pc", [PC, 128, C], BF16, kind="ExternalInput")
    out_d = nc.dram_tensor("out", [P, N], BF16, kind="ExternalOutput")
    ones_d = nc.inline_tensor(np.ones((128, 128), dtype=NPBF16), "ones_mat")

    IDENT = mybir.ActivationFunctionType.Identity
    SQUARE = mybir.ActivationFunctionType.Square
    MULT = mybir.AluOpType.mult

    with tile.TileContext(nc) as tc:
        with (
            tc.tile_pool(name="resident", bufs=1) as rpool,
            tc.tile_pool(name="x2p", bufs=2) as x2_pool,
            tc.tile_pool(name="sqp", bufs=2) as sq_pool,
            tc.tile_pool(name="up", bufs=2) as u_pool,
            tc.tile_pool(name="op", bufs=3) as o_pool,
            tc.tile_pool(name="pm", bufs=1, space=bass.MemorySpace.PSUM) as pm_pool,
        ):
            # ---- resident tiles ----
            x8_sb = rpool.tile([128, KC, N], FP8, tag="x8sb", name="x8sb")
            w8_sb = rpool.tile([128, KC, P], FP8, tag="w8sb", name="w8sb")
            ones_sb = rpool.tile([128, 128], BF16, tag="ones", name="ones_sb")
            i2r = rpool.tile([128, N], BF16, tag="i2r", name="i2r")
            w2col = rpool.tile([128, PC], F32, tag="w2col", name="w2col")
            wpc_t = [
                rpool.tile([128, C], BF16, tag=f"wpc{i}", name=f"wpc{i}")
                for i in range(PC)
            ]

            # whole PSUM as one 8-bank ring; each slot = one bank, FD used
            ps_all = pm_pool.tile(
                [128, 8, FD], F32, padded_shape=[128, 8, 512], name="ps_all"
            )

            # ---- input DMAs, all on the sync HWDGE ring ----
            nc.sync.dma_start(ones_sb[:], ones_d[:])
            for k in range(2):
                nc.sync.dma_start(w8_sb[:, k, :], w8_d[k])
            # x fp8 in (k, third) pieces: squares + first matmuls start early
            T0, T1 = 1024, 2048
            for k in range(KC):
                nc.sync.dma_start(x8_sb[:, k, 0:T0], x8_d[k, :, 0:T0])
            for i in range(2):
                nc.sync.dma_start(wpc_t[i][:], wpc_d[i])
            for k in range(KC):
                nc.sync.dma_start(x8_sb[:, k, T0:T1], x8_d[k, :, T0:T1])
            for k in range(2, KC):
                nc.sync.dma_start(w8_sb[:, k, :], w8_d[k])
            for k in range(KC):
                nc.sync.dma_start(x8_sb[:, k, T1:N], x8_d[k, :, T1:N])
            for i in range(2, PC):
                nc.sync.dma_start(wpc_t[i][:], wpc_d[i])

            def w2_op(i):
                sq = sq_pool.tile([128, C], BF16, tag="sq", name="sq")
                nc.vector.scalar_tensor_tensor(
                    sq[:], wpc_t[i][:], 1.0, wpc_t[i][:],
                    op0=MULT, op1=MULT,
                    accum_out=w2col[:, i:i + 1],
                )

            # ---- i2 phase: x^2 + ones-matmul broadcast, slots 0..6 ----
            for c in range(NCH):
                sl = slice(FD * c, FD * (c + 1))
                x2t = x2_pool.tile([128, KC, FD], BF16, tag="x2t", name="x2t")
                if c < 4:
                    nc.scalar.activation(x2t[:], x8_sb[:, :, sl], SQUARE)
                else:
                    nc.vector.tensor_mul(
                        x2t[:], x8_sb[:, :, sl], x8_sb[:, :, sl]
                    )
                for k in range(KC):
                    nc.tensor.matmul(
                        ps_all[:, c, :], ones_sb[:], x2t[:, k, :],
                        start=(k == 0), stop=(k == KC - 1),
                    )
                nc.scalar.copy(i2r[:, sl], ps_all[:, c, :])

            w2_op(0)
            w2_op(1)

            # ---- main loop over p-chunks ----
            for pc in range(PC):
                M = min(128, P - 128 * pc)
                psl = slice(128 * pc, 128 * pc + M)
                slots = [(NCH + NCH * pc + t) % 8 for t in range(NCH)]

                # pass1/pass2: fp8 DoubleRow accumulate of -2*x.w
                for t in range(NCH):
                    nsl = slice(FD * t, FD * (t + 1))
                    nc.tensor.matmul(
                        ps_all[:M, slots[t], :],
                        w8_sb[:, 0:2, psl], x8_sb[:, 0:2, nsl],
                        start=True, stop=False, perf_mode=DR,
                    )
                for t in range(NCH):
                    nsl = slice(FD * t, FD * (t + 1))
                    nc.tensor.matmul(
                        ps_all[:M, slots[t], :],
                        w8_sb[:, 2:4, psl], x8_sb[:, 2:4, nsl],
                        start=False, stop=True, perf_mode=DR,
                    )

                # spread the w2 producers across the loop
                if pc + 2 < PC:
                    w2_op(pc + 2)

                u = u_pool.tile([128, N], BF16, tag="u", name="u")
                o = o_pool.tile([128, N], BF16, tag="o", name="o")
                u3 = u.rearrange("p (c f) -> p c f", f=FD)

                # A: evict psum -> u (+w2[p]).  c0 on DVE; c1..c6 ScalarE
                for s0, ln, off in _runs(slots[0:DVE_CH]):
                    nc.vector.tensor_scalar_add(
                        u3[:M, off:off + ln, :],
                        ps_all[:M, s0:s0 + ln, :],
                        w2col[:M, pc:pc + 1],
                    )
                for grp_base, grp_end in ((DVE_CH, 4), (4, NCH)):
                    grp = slots[grp_base:grp_end]
                    for s0, ln, off in _runs(grp):
                        off += grp_base
                        nc.scalar.activation(
                            u3[:M, off:off + ln, :],
                            ps_all[:M, s0:s0 + ln, :],
                            IDENT, bias=w2col[:M, pc:pc + 1], scale=1.0,
                        )

                # B: o = u + i2r  (bf16 2x)
                nc.vector.tensor_add(o[:M, :], u[:M, :], i2r[:M, :])

                # one big out-DMA on the gpsimd SWDGE ring (6272B lines)
                nc.gpsimd.dma_start(out_d[psl, :], o[:M, :])

    nc.compile()
    return nc


def _get_nc():
    if "nc" not in _CACHE:
        _CACHE["nc"] = _build()
    return _CACHE["nc"]


def _make_in_maps(input, weights):
    x = np.asarray(input, dtype=np.float32)
    w = np.asarray(weights, dtype=np.float32).reshape(P, C)

    wT = np.ascontiguousarray((-2.0 * w).T)               # [C, P]
    w8 = wT.astype(NPFP8).reshape(KC, 128, P)
    w_pad = np.zeros((P_PAD, C), np.float32)
    w_pad[:P] = w
    w_pc = w_pad.astype(NPBF16).reshape(PC, 128, C)

    in_maps = []
    for c in range(NCORES):
        sh = x[c * BL:(c + 1) * BL]                       # [4, 512, 28, 28]
        xT = np.ascontiguousarray(sh.transpose(1, 0, 2, 3).reshape(C, N))
        x8 = xT.astype(NPFP8).reshape(KC, 128, N)
        in_maps.append({"x8": x8, "w8": w8, "w_pc": w_pc})
    return in_maps


def run(input, weights, trace=False):
    """Returns (output [32,2000,28,28] f32, BassKernelResults)."""
    nc = _get_nc()
    in_maps = _make_in_maps(input, weights)
    res = bass_utils.run_bass_kernel_spmd(
        nc, in_maps, core_ids=list(range(NCORES)), trace=trace
    )
    # per-core out is [P, N] = [P, BL, HW]; reorder to [BL, P, HW]
    outs = [
        res.results[c]["out"].reshape(P, BL, HW).transpose(1, 0, 2)
        for c in range(NCORES)
    ]
    out = (
        np.concatenate(outs, axis=0).astype(np.float32).reshape(B, P, H, W)
    )
    return out, res


def kernel(input, weights):
    out, _ = run(input, weights, trace=False)
    return out
